# revision 11
# baseline (speedup 1.0000x reference)
"""Trainium2 Bass kernel for a 2-layer GCN (EnhancedHockeyGNN).

Strategy (8 NeuronCores, SPMD):
  - Nodes sharded row-wise across cores (dst-ownership); small weights replicated.
  - Per layer: xs = (x @ W) * dinv[src-side] computed on the owning core,
    AllGather'd into a full table in each core's DRAM.
  - Edges (incl. explicit self-loops) are sharded by dst owner, bin-packed into
    groups of <=128 dst nodes / <=2048 edges. Per 128-edge tile the src rows are
    fetched with an indirect DMA (one row per partition), and a one-hot matrix
    (value dinv[dst], built on the vector engine) turns the segment-sum into a
    PSUM-accumulated matmul chain producing feature-major aggregates.
  - BN+ReLU (eval mode) folds to a per-channel affine -> single scalar-engine
    activation per group.
  - Readout computes logits + log_softmax for every node; the host selects the
    requested game_indices rows (pure index routing).
"""
import math

import numpy as np

# ---------------------------------------------------------------- constants
N = 100000
E_RAW = 1600000
F_IN = 128
H = 128
NC = 8
SHARD = 12544            # multiple of 128; 8 * 12544 = 100352 >= N
NPAD = NC * SHARD
GROUP_EDGES = 2048       # edges per group (16 tiles of 128)
GROUP_TILES = GROUP_EDGES // 128
GROUP_DSTS = 128         # max dst nodes per group (PSUM free dim)
EPS = 1e-5

_CACHE = {}


# ---------------------------------------------------------------- host prep
def _bin_pack(counts, G):
    """Pack dst nodes (with edge `counts`) into G bins with <=GROUP_DSTS nodes
    and <=GROUP_EDGES edges. Returns (group_of, pos_in_group) or None."""
    order = np.argsort(-counts, kind="stable")
    bin_edges = np.zeros(G, dtype=np.int64)
    bin_nodes = np.zeros(G, dtype=np.int64)
    group_of = np.full(counts.shape[0], -1, dtype=np.int32)
    pos_in_group = np.full(counts.shape[0], -1, dtype=np.int32)
    # simple first-fit on the (always feasible in practice) sorted order
    for d in order:
        c = counts[d]
        placed = False
        for b in range(G):
            if bin_edges[b] + c <= GROUP_EDGES and bin_nodes[b] < GROUP_DSTS:
                group_of[d] = b
                pos_in_group[d] = bin_nodes[b]
                bin_edges[b] += c
                bin_nodes[b] += 1
                placed = True
                break
        if not placed:
            return None
    return group_of, pos_in_group


def _prepare(x, edge_index, cfg):
    """All numpy. Returns per-core input tables + readout mapping."""
    n, npad, shard, nc = cfg["N"], cfg["NPAD"], cfg["SHARD"], cfg["NC"]
    ge, gt = cfg["GROUP_EDGES"], cfg["GROUP_TILES"]

    src = np.asarray(edge_index[0], dtype=np.int64)
    dst = np.asarray(edge_index[1], dtype=np.int64)
    deg = np.bincount(dst, minlength=n).astype(np.float64) + 1.0
    dinv = (1.0 / np.sqrt(deg)).astype(np.float32)
    dinv_pad_full = np.ones(npad, dtype=np.float32)
    dinv_pad_full[:n] = dinv

    # append self loops
    sall = np.concatenate([src, np.arange(n, dtype=np.int64)])
    dall = np.concatenate([dst, np.arange(n, dtype=np.int64)])
    owner = dall // shard

    # per-core packing
    packs = []
    Es = []
    for c in range(nc):
        m = owner == c
        Es.append(int(m.sum()))
    G = max(int(math.ceil(e / ge)) for e in Es)
    while True:
        packs = []
        ok = True
        for c in range(nc):
            m = owner == c
            d0 = (dall[m] - c * shard).astype(np.int64)
            counts = np.bincount(d0, minlength=shard)
            r = _bin_pack(counts, G)
            if r is None:
                ok = False
                break
            packs.append((r[0], r[1], d0, sall[m]))
        if ok:
            break
        G += 1

    ntiles = G * gt
    # padded-row index of every (real) node in the xs2 table layout
    padded_row = np.zeros(npad, dtype=np.int64)
    for c in range(nc):
        group_of, pos, _, _ = packs[c]
        rows = c * G * 128 + group_of.astype(np.int64) * 128 + pos.astype(np.int64)
        base = c * shard
        padded_row[base:base + shard] = rows

    per_core = []
    for c in range(nc):
        group_of, pos, d0, s_nodes = packs[c]
        g_of_edge = group_of[d0]
        order = np.argsort(g_of_edge, kind="stable")
        d0o, so, go = d0[order], s_nodes[order], g_of_edge[order]
        src1 = np.zeros((128, ntiles), dtype=np.int32)
        src2 = np.zeros((128, ntiles), dtype=np.int32)
        dloc = np.full((128, ntiles), 300.0, dtype=np.float32)
        dinv_dst = np.zeros((128, ntiles), dtype=np.float32)
        gstart = np.searchsorted(go, np.arange(G))
        gend = np.searchsorted(go, np.arange(G) + 1)
        for g in range(G):
            a, b = int(gstart[g]), int(gend[g])
            k = b - a
            assert k <= ge
            sl_s = so[a:b]
            sl_d = d0o[a:b]
            t = np.arange(k) // 128
            p = np.arange(k) % 128
            cols = g * gt + t
            src1[p, cols] = sl_s
            src2[p, cols] = padded_row[sl_s + 0]
            dloc[p, cols] = pos[sl_d]
            dinv_dst[p, cols] = dinv_pad_full[c * shard + sl_d]
        # dinv in natural order for the xs pre-scale
        jj = np.arange(shard)
        dinv_nat = dinv_pad_full[c * shard + jj].reshape(shard // 128, 128).T.copy()
        # x shard, transposed (feature-major) for stationary loads
        xs_shape = np.zeros((shard, x.shape[1]), dtype=np.float32)
        lo, hi = c * shard, min((c + 1) * shard, n)
        xs_shape[: hi - lo] = x[lo:hi]
        xT = np.ascontiguousarray(xs_shape.T)
        # dinv for padded layout (xs2 pre-scale): row (g, p) -> dst node
        dinv_padlay = np.zeros((128, G), dtype=np.float32)
        inv_nodes = np.full(G * 128, -1, dtype=np.int64)
        inv_nodes[group_of.astype(np.int64) * 128 + pos.astype(np.int64)] = np.arange(shard)
        valid = inv_nodes >= 0
        vals = np.zeros(G * 128, dtype=np.float32)
        vals[valid] = dinv_pad_full[c * shard + inv_nodes[valid]]
        dinv_padlay[:, :] = vals.reshape(G, 128).T
        per_core.append(dict(src1=src1, src2=src2, dloc=dloc, dinv_dst=dinv_dst,
                             dinv_nat=dinv_nat, dinv_padlay=dinv_padlay, xT=xT))
    return per_core, padded_row, G, ntiles


def _fold_bn(gamma, beta, mean, var, b):
    s = (gamma / np.sqrt(var + EPS)).astype(np.float32)
    t = ((b - mean) * s + beta).astype(np.float32)
    return s.reshape(H, 1), t.reshape(H, 1)


# ---------------------------------------------------------------- bass build
def _build(cfg, G, ntiles, part):
    """part='a': xs1 + AG + layer1 + xs2-shard out. part='b': AG xs2 + layer2 + readout."""
    import concourse.bacc as bacc
    import concourse.bass as bass
    import concourse.mybir as mybir
    import concourse.tile as tile

    fp32 = mybir.dt.float32
    i32 = mybir.dt.int32
    AF = mybir.ActivationFunctionType

    nc_ = cfg["NC"]
    shard = cfg["SHARD"]
    npad = cfg["NPAD"]
    gt = cfg["GROUP_TILES"]
    ntile_nat = shard // 128
    h = cfg["H"]
    fin = cfg["F_IN"]

    nc = bacc.Bacc(None, target_bir_lowering=False, debug=False, num_devices=nc_)

    # common inputs
    iota_in = nc.dram_tensor("iota", [128, 128], fp32, kind="ExternalInput")
    dloc_in = nc.dram_tensor("dloc", [128, ntiles], fp32, kind="ExternalInput")
    dd_in = nc.dram_tensor("dinv_dst", [128, ntiles], fp32, kind="ExternalInput")

    if part == "a":
        xT_in = nc.dram_tensor("xT", [fin, shard], fp32, kind="ExternalInput")
        w1_in = nc.dram_tensor("W1", [fin, h], fp32, kind="ExternalInput")
        w2_in = nc.dram_tensor("W2", [h, h], fp32, kind="ExternalInput")
        s1_in = nc.dram_tensor("s1", [h, 1], fp32, kind="ExternalInput")
        t1_in = nc.dram_tensor("t1", [h, 1], fp32, kind="ExternalInput")
        src1_in = nc.dram_tensor("src1", [128, ntiles], i32, kind="ExternalInput")
        dn_in = nc.dram_tensor("dinv_nat", [128, ntile_nat], fp32, kind="ExternalInput")
        dp_in = nc.dram_tensor("dinv_padlay", [128, G], fp32, kind="ExternalInput")
        out_xs2 = nc.dram_tensor("xs2_shard_out", [G * 128, h], fp32,
                                 kind="ExternalOutput")
    else:
        xs2_in = nc.dram_tensor("xs2_shard_in", [G * 128, h], fp32,
                                kind="ExternalInput")
        wf_in = nc.dram_tensor("Wf", [h, 2], fp32, kind="ExternalInput")
        bf_in = nc.dram_tensor("bf_rep", [128, 2], fp32, kind="ExternalInput")
        s2_in = nc.dram_tensor("s2", [h, 1], fp32, kind="ExternalInput")
        t2_in = nc.dram_tensor("t2", [h, 1], fp32, kind="ExternalInput")
        src2_in = nc.dram_tensor("src2", [128, ntiles], i32, kind="ExternalInput")
        out_lp = nc.dram_tensor("logp", [128, 2 * G], fp32, kind="ExternalOutput")

    with tile.TileContext(nc) as tc:
        with (
            tc.tile_pool(name="res", bufs=1) as res,
            tc.tile_pool(name="big", bufs=1) as big,
            tc.tile_pool(name="stream", bufs=1) as st,
            tc.tile_pool(name="ps", bufs=1, space="PSUM") as ps,
            tc.tile_pool(name="dram", bufs=1, space="DRAM") as dram,
        ):
            iota_t = res.tile([128, 128], fp32)
            dloc_t = res.tile([128, ntiles], fp32)
            dd_t = res.tile([128, ntiles], fp32)
            nc.sync.dma_start(out=iota_t[:], in_=iota_in[:])
            nc.sync.dma_start(out=dloc_t[:], in_=dloc_in[:])
            nc.sync.dma_start(out=dd_t[:], in_=dd_in[:])

            def stage_xs(lhsT_ap, n_node_tiles, w_tile, dinv_t, bounce):
                xsb = big.tile([128, G * 128], fp32, name="xsb", tag="big_a")
                for j in range(n_node_tiles):
                    pxs = ps.tile([128, h], fp32, name="pxs", tag="pxs", bufs=2)
                    nc.tensor.matmul(pxs[:], lhsT_ap(j), w_tile[:], start=True,
                                     stop=True)
                    nc.vector.tensor_scalar(
                        out=xsb[:, j * 128:(j + 1) * 128],
                        in0=pxs[:],
                        scalar1=dinv_t[:, j:j + 1],
                        scalar2=None,
                        op0=mybir.AluOpType.mult,
                    )
                dest = bass.AP(bounce[:].tensor, 0,
                               [[h, 128], [128 * h, n_node_tiles], [1, h]])
                nc.sync.dma_start(out=dest, in_=xsb[:].rearrange(
                    "p (j f) -> p j f", f=h)[:, :n_node_tiles, :])

            def edge_layer(src_t, xs_full_ap, s_t, t_t, hT):
                for g in range(G):
                    oh = st.tile([128, gt, 128], fp32, name="oh", tag="oh", bufs=2)
                    nc.vector.tensor_tensor(
                        out=oh[:],
                        in0=dloc_t[:, g * gt:(g + 1) * gt].to_broadcast(
                            [128, gt, 128]),
                        in1=bass.AP(iota_t[:].tensor, iota_t[:].offset,
                                    [iota_t[:].ap[0], [0, gt], [1, 128]]),
                        op=mybir.AluOpType.is_equal,
                    )
                    nc.vector.tensor_tensor(
                        out=oh[:],
                        in0=oh[:],
                        in1=dd_t[:, g * gt:(g + 1) * gt].to_broadcast(
                            [128, gt, 128]),
                        op=mybir.AluOpType.mult,
                    )
                    pg = ps.tile([h, 128], fp32, name="pg", tag="pg", bufs=4)
                    for t in range(gt):
                        k = g * gt + t
                        msg = st.tile([128, h], fp32, name="msg", tag="msg",
                                      bufs=8)
                        nc.gpsimd.indirect_dma_start(
                            out=msg[:],
                            out_offset=None,
                            in_=xs_full_ap,
                            in_offset=bass.IndirectOffsetOnAxis(
                                ap=src_t[:, k:k + 1], axis=0),
                        )
                        nc.tensor.matmul(pg[:], msg[:], oh[:, t, :],
                                         start=(t == 0), stop=(t == gt - 1))
                    nc.scalar.activation(
                        out=hT[:, g * 128:(g + 1) * 128], in_=pg[:],
                        func=AF.Relu, bias=t_t[:], scale=s_t[:],
                    )

            if part == "a":
                w1_t = res.tile([fin, h], fp32)
                w2_t = res.tile([h, h], fp32)
                s1_t = res.tile([h, 1], fp32)
                t1_t = res.tile([h, 1], fp32)
                src1_t = res.tile([128, ntiles], i32)
                dn_t = res.tile([128, ntile_nat], fp32)
                dp_t = res.tile([128, G], fp32)
                for t_, i_ in ((w1_t, w1_in), (w2_t, w2_in), (s1_t, s1_in),
                               (t1_t, t1_in), (src1_t, src1_in), (dn_t, dn_in),
                               (dp_t, dp_in)):
                    nc.sync.dma_start(out=t_[:], in_=i_[:])

                xs1_shard = dram.tile([shard, h], fp32)
                xs1_full = dram.tile([npad, h], fp32, addr_space="Shared")

                def desc_a(j):
                    t = st.tile([128, 128], fp32, name="xTt", tag="lhsT", bufs=4)
                    nc.sync.dma_start(out=t[:],
                                      in_=xT_in[:, j * 128:(j + 1) * 128])
                    return t[:]

                stage_xs(desc_a, ntile_nat, w1_t, dn_t, xs1_shard)
                nc.gpsimd.collective_compute(
                    "AllGather", mybir.AluOpType.bypass,
                    replica_groups=[list(range(nc_))],
                    ins=[xs1_shard[:].opt()], outs=[xs1_full[:].opt()],
                )
                hT = big.tile([128, G * 128], fp32, name="hT", tag="big_b")
                edge_layer(src1_t, xs1_full[:], s1_t, t1_t, hT)
                stage_xs(lambda j: hT[:, j * 128:(j + 1) * 128], G, w2_t, dp_t,
                         out_xs2)
            else:
                wf_t = res.tile([h, 2], fp32)
                bf_t = res.tile([128, 2], fp32)
                s2_t = res.tile([h, 1], fp32)
                t2_t = res.tile([h, 1], fp32)
                src2_t = res.tile([128, ntiles], i32)
                for t_, i_ in ((wf_t, wf_in), (bf_t, bf_in), (s2_t, s2_in),
                               (t2_t, t2_in), (src2_t, src2_in)):
                    nc.sync.dma_start(out=t_[:], in_=i_[:])

                xs2_shard = dram.tile([G * 128, h], fp32)
                xs2_full = dram.tile([nc_ * G * 128, h], fp32,
                                     addr_space="Shared")
                nc.gpsimd.dma_start(out=xs2_shard[:], in_=xs2_in[:])
                nc.gpsimd.collective_compute(
                    "AllGather", mybir.AluOpType.bypass,
                    replica_groups=[list(range(nc_))],
                    ins=[xs2_shard[:].opt()], outs=[xs2_full[:].opt()],
                )
                h2T = big.tile([128, G * 128], fp32, name="h2T", tag="big_b")
                edge_layer(src2_t, xs2_full[:], s2_t, t2_t, h2T)

                lg = res.tile([128, 2 * G], fp32)
                for j in range(G):
                    plg = ps.tile([128, 2], fp32, name="plg", tag="plg", bufs=2)
                    nc.tensor.matmul(plg[:], h2T[:, j * 128:(j + 1) * 128],
                                     wf_t[:], start=True, stop=True)
                    nc.vector.tensor_add(out=lg[:, 2 * j:2 * j + 2],
                                         in0=plg[:], in1=bf_t[:])

                def strided(base, start):
                    a = base[:]
                    return bass.AP(a.tensor, a.offset + start,
                                   [a.ap[0], [2, G]])

                z0, z1 = strided(lg, 0), strided(lg, 1)
                mx = res.tile([128, G], fp32)
                nc.vector.tensor_tensor(out=mx[:], in0=z0, in1=z1,
                                        op=mybir.AluOpType.max)
                sm0 = res.tile([128, G], fp32)
                sm1 = res.tile([128, G], fp32)
                nc.vector.tensor_sub(out=sm0[:], in0=z0, in1=mx[:])
                nc.vector.tensor_sub(out=sm1[:], in0=z1, in1=mx[:])
                e0 = res.tile([128, G], fp32)
                e1 = res.tile([128, G], fp32)
                nc.scalar.activation(out=e0[:], in_=sm0[:], func=AF.Exp)
                nc.scalar.activation(out=e1[:], in_=sm1[:], func=AF.Exp)
                se = res.tile([128, G], fp32)
                nc.vector.tensor_add(out=se[:], in0=e0[:], in1=e1[:])
                ls = res.tile([128, G], fp32)
                nc.scalar.activation(out=ls[:], in_=se[:], func=AF.Ln)
                nc.vector.tensor_sub(out=sm0[:], in0=sm0[:], in1=ls[:])
                nc.vector.tensor_sub(out=sm1[:], in0=sm1[:], in1=ls[:])
                lpo = res.tile([128, 2 * G], fp32)
                nc.vector.tensor_copy(out=strided(lpo, 0), in_=sm0[:])
                nc.vector.tensor_copy(out=strided(lpo, 1), in_=sm1[:])
                nc.sync.dma_start(out=out_lp[:], in_=lpo[:])

    nc.compile()
    return nc


# ---------------------------------------------------------------- main entry
def _run(x, edge_index, game_indices,
         W1, b1, g1, be1, m1, v1, W2, b2, g2, be2, m2, v2, Wf, bf,
         trace=False, cfg=None):
    from concourse import bass_utils

    if cfg is None:
        cfg = dict(N=N, NPAD=NPAD, SHARD=SHARD, NC=NC, GROUP_EDGES=GROUP_EDGES,
                   GROUP_TILES=GROUP_TILES, H=H, F_IN=F_IN)

    x = np.asarray(x, dtype=np.float32)
    key = ("prep", x.shape, int(np.asarray(edge_index)[0, 0]),
           int(np.asarray(edge_index).sum() % (1 << 31)))
    if key in _CACHE:
        per_core, padded_row, G, ntiles = _CACHE[key]
    else:
        per_core, padded_row, G, ntiles = _prepare(x, np.asarray(edge_index), cfg)
        _CACHE.clear()
        _CACHE[key] = (per_core, padded_row, G, ntiles)

    bkey = ("bass", G, ntiles)
    if bkey in _CACHE:
        nc_a, nc_b = _CACHE[bkey]
    else:
        nc_a = _build(cfg, G, ntiles, "a")
        nc_b = _build(cfg, G, ntiles, "b")
        _CACHE[bkey] = (nc_a, nc_b)

    s1, t1 = _fold_bn(np.asarray(g1), np.asarray(be1), np.asarray(m1),
                      np.asarray(v1), np.asarray(b1))
    s2, t2 = _fold_bn(np.asarray(g2), np.asarray(be2), np.asarray(m2),
                      np.asarray(v2), np.asarray(b2))
    iota = np.broadcast_to(np.arange(128, dtype=np.float32), (128, 128)).copy()
    bf_rep = np.broadcast_to(np.asarray(bf, dtype=np.float32), (128, 2)).copy()

    ncores = cfg["NC"]
    in_maps_a = []
    for c in range(ncores):
        pc = per_core[c]
        in_maps_a.append(dict(
            xT=pc["xT"], W1=np.asarray(W1, np.float32),
            W2=np.asarray(W2, np.float32), s1=s1, t1=t1, iota=iota,
            src1=pc["src1"], dloc=pc["dloc"], dinv_dst=pc["dinv_dst"],
            dinv_nat=pc["dinv_nat"], dinv_padlay=pc["dinv_padlay"],
        ))
    res_a = bass_utils.run_bass_kernel_spmd(
        nc_a, in_maps_a, core_ids=list(range(ncores)), trace=trace)

    in_maps_b = []
    for c in range(ncores):
        pc = per_core[c]
        in_maps_b.append(dict(
            xs2_shard_in=res_a.results[c]["xs2_shard_out"],
            Wf=np.asarray(Wf, np.float32), bf_rep=bf_rep, s2=s2, t2=t2,
            iota=iota, src2=pc["src2"], dloc=pc["dloc"],
            dinv_dst=pc["dinv_dst"],
        ))
    res_b = bass_utils.run_bass_kernel_spmd(
        nc_b, in_maps_b, core_ids=list(range(ncores)), trace=trace)

    class _Res:
        pass

    res = _Res()
    res.results = res_b.results
    res.exec_time_ns = ((res_a.exec_time_ns or 0) + (res_b.exec_time_ns or 0)) \
        if (res_a.exec_time_ns or res_b.exec_time_ns) else None
    res.parts = (res_a, res_b)

    # assemble: logp rows for requested game indices
    gi = np.asarray(game_indices, dtype=np.int64)
    rows = padded_row[gi]                      # padded position of each node
    c_idx = rows // (G * 128)
    j_idx = (rows % (G * 128)) // 128
    p_idx = rows % 128
    lp = np.stack([res.results[c]["logp"] for c in range(ncores)])
    out = np.empty((gi.shape[0], 2), dtype=np.float32)
    out[:, 0] = lp[c_idx, p_idx, 2 * j_idx]
    out[:, 1] = lp[c_idx, p_idx, 2 * j_idx + 1]
    return out, res


def kernel(**inputs):
    out, _ = _run(**inputs)
    return out


def kernel_profiled(**inputs):
    out, res = _run(**inputs, trace=True)
    return out, res
